# revision 16
# baseline (speedup 1.0000x reference)
"""Multi-head self-attention (B=4, T=2048, D=1024, H=16) on 8 TRN2 NeuronCores.

Sharding: core c = 2*b + j computes batch b, heads j*8..j*8+7 (tensor-parallel
over heads), and a partial projection over its 512 attention-output columns.
The host sums the two partial projections per batch. No collectives.

Per-core dataflow (fp16 operands, fp32 PSUM):
  - Scores in transposed layout s^T[k, q] with PE row-tiling: the two heads
    of a pair contract K=64 each on array rows 0:64 / 64:128 concurrently
    (tile_position (0,0)/(64,0)) -> 2x score throughput vs zero-padding.
  - One 2048-wide exp per k-tile on ScalarE covers both heads (scale=1/8).
  - attn@V with PE col-tiling: the pair's V matmuls write po[0:64]/po[64:128]
    concurrently (tile_position (0,0)/(0,64)) -> 2x vs M=65 serial.
  - Softmax denominators: DVE chain-sums the 16 exp tiles (fp16, 2x mode),
    then two M=1 ones-matmuls (col positions 0/32) reduce partitions,
    reciprocal_approx_fast + gpsimd partition_broadcast + one DVE multiply
    normalize into ats (fp16).
  - QKV / V / projection matmul chunks are interleaved into the attention
    iterations so the PE uses the slack under the ScalarE-bound exp corridor.
"""

import os

import numpy as np

import concourse.mybir as mybir
from concourse import bacc, bass_isa
from concourse.tile import TileContext
from concourse.bass_utils import run_bass_kernel_spmd

B, T, D, H = 4, 2048, 1024, 16
HD = D // H
SCALE = HD**-0.5
P = 128
F16 = mybir.dt.float16
F32 = mybir.dt.float32

LAST_RESULT = None
_built = None


def _ensure_ntff_hook():
    """Install the axon NTFF profile hook if the env lacks antenv.axon_hooks."""
    try:
        import antenv.axon_hooks  # noqa: F401

        return
    except ImportError:
        pass
    try:
        import sys
        import types

        import antenv
        from trn_agent_boot.trn_boot import _ntff_profile_via_ctypes

        hook = _ntff_profile_via_ctypes("/opt/axon/libaxon_pjrt.so")
        if hook is None:
            return
        mod = types.ModuleType("antenv.axon_hooks")
        mod._hook = hook
        mod.get_axon_ntff_profile_hook = lambda: mod._hook

        def _set(h):
            mod._hook = h

        mod.set_axon_ntff_profile_hook = _set
        sys.modules["antenv.axon_hooks"] = mod
        antenv.axon_hooks = mod
    except Exception:
        pass


def _build():
    nc = bacc.Bacc("TRN2", target_bir_lowering=False, debug=False, num_devices=8)

    xT = nc.dram_tensor("xT", [D, T], F16, kind="ExternalInput")  # x[b].T
    wqkT = nc.dram_tensor("wqkT", [D, 1024], F16, kind="ExternalInput")  # (q|k).T
    wvT = nc.dram_tensor("wvT", [D, 512], F16, kind="ExternalInput")
    wpT = nc.dram_tensor("wpT", [512, D], F16, kind="ExternalInput")
    qkb = nc.dram_tensor("qkb", [1024], F32, kind="ExternalInput")
    vb = nc.dram_tensor("vb", [512], F16, kind="ExternalInput")
    pb = nc.dram_tensor("pb", [D], F32, kind="ExternalInput")
    yT = nc.dram_tensor("yT", [D, T], F16, kind="ExternalOutput")

    Exp = mybir.ActivationFunctionType.Exp
    mult = mybir.AluOpType.mult
    add = mybir.AluOpType.add

    with TileContext(nc) as tc:
        with (
            tc.tile_pool(name="pers", bufs=1) as pers,
            tc.tile_pool(name="e2p", bufs=1) as e2p,
            tc.tile_pool(name="ph1", bufs=1) as ph1,
            tc.tile_pool(name="ph3", bufs=1) as ph3,
            tc.tile_pool(name="outp", bufs=1) as outp,
            tc.tile_pool(name="mpool", bufs=1, space="PSUM") as mpool,
            tc.tile_pool(name="dpool", bufs=1, space="PSUM") as dpool,
        ):
            # ---- persistent tensors ----
            # K^T per pair: even head feats on partitions 0:64, odd on 64:128
            kts = [pers.tile([P, T], F16, tag=f"kt{i}", name=f"kt{i}") for i in range(4)]
            # Q^T per pair, same partition split
            qpair = [pers.tile([P, T], F16, tag=f"qp{i}", name=f"qp{i}") for i in range(4)]
            # normalized attention out per pair (rows 0:64 even, 64:128 odd)
            ats = [pers.tile([P, T], F16, tag=f"at{i}", name=f"at{i}") for i in range(4)]
            V4 = pers.tile([P, 16, 8, HD], F16, tag="v4", name="v4")  # [t-part, tt, head, hd]
            acc = pers.tile([P, T], F16, tag="acc", name="acc")  # denom partials (A|B cols)
            rb = pers.tile([P, 1024], F32, tag="rb", name="rb")  # 1/d broadcast
            ones = pers.tile([P, 64], F16, tag="ones", name="ones")
            qkb_sb = pers.tile([P, 8], F32, tag="qkb", name="qkb")
            vb_sb = pers.tile([P, 512], F16, tag="vb", name="vb")
            pb_sb = pers.tile([P, 8], F32, tag="pb", name="pb")
            # e2[:, kt, 0:1024] = exp scores head A, [:, kt, 1024:2048] head B
            e2 = e2p.tile([P, 16, 2048], F16, tag="e2", name="e2")

            # ---- input loads ----
            xts = []
            wqks = []
            for dt in range(8):
                w_ = ph1.tile([P, 1024], F16, tag=f"wqk{dt}", name=f"wqk{dt}")
                nc.sync.dma_start(w_[:], wqkT.ap()[dt * P : (dt + 1) * P, :])
                wqks.append(w_)
            for dt in range(8):
                t_ = ph1.tile([P, T], F16, tag=f"xt{dt}", name=f"xt{dt}")
                nc.sync.dma_start(t_[:], xT.ap()[dt * P : (dt + 1) * P, :])
                xts.append(t_)
            wvs = []
            for dt in range(8):
                w_ = ph1.tile([P, 512], F16, tag=f"wv{dt}", name=f"wv{dt}")
                nc.sync.dma_start(w_[:], wvT.ap()[dt * P : (dt + 1) * P, :])
                wvs.append(w_)
            wps = []
            for dt in range(4):
                w_ = ph3.tile([P, D], F16, tag=f"wp{dt}", name=f"wp{dt}")
                nc.sync.dma_start(w_[:], wpT.ap()[dt * P : (dt + 1) * P, :])
                wps.append(w_)
            nc.sync.dma_start(qkb_sb[:], qkb.rearrange("(o p) -> p o", p=P))
            nc.sync.dma_start(vb_sb[:], vb.ap()[None, :].to_broadcast((P, 512)))
            nc.sync.dma_start(pb_sb[:], pb.rearrange("(o p) -> p o", p=P))
            nc.vector.memset(ones[:], 1.0)
            # warm the Exp table while phase 1 runs
            nc.scalar.activation(rb[0:1, 0:1], qkb_sb[0:1, 0:1], Exp, scale=0.0)

            # ---- helper: QK projection chunk (ft, tcc, half) ----
            # ft 0..3 -> qpair[ft]; ft 4..7 -> kts[ft-4]
            def qk_chunk(ft, tcc, half):
                mp = mpool.tile([P, 512], F32, tag="mp", name="mp")
                col = tcc * 1024 + half * 512
                for dt in range(8):
                    nc.tensor.matmul(
                        mp[:],
                        lhsT=wqks[dt][:, ft * P : (ft + 1) * P],
                        rhs=xts[dt][:, col : col + 512],
                        start=(dt == 0),
                        stop=(dt == 7),
                    )
                dst = qpair[ft] if ft < 4 else kts[ft - 4]
                nc.vector.tensor_scalar_add(
                    dst[:, col : col + 512], mp[:], qkb_sb[:, ft : ft + 1]
                )

            def v_chunk(tt):
                mp = mpool.tile([P, 512], F32, tag="mp", name="mp")
                for dt in range(8):
                    nc.tensor.matmul(
                        mp[:],
                        lhsT=xts[dt][:, tt * P : (tt + 1) * P],
                        rhs=wvs[dt][:],
                        start=(dt == 0),
                        stop=(dt == 7),
                    )
                nc.vector.tensor_tensor(
                    V4[:, tt, :, :],
                    mp.rearrange("p (h e) -> p h e", e=HD),
                    vb_sb.rearrange("p (h e) -> p h e", e=HD),
                    add,
                )

            def proj_chunk(et, tcc, half, obs):
                mp = mpool.tile([P, 512], F32, tag="mp", name="mp")
                col = tcc * 1024 + half * 512
                for dt in range(4):
                    nc.tensor.matmul(
                        mp[:],
                        lhsT=wps[dt][:, et * P : (et + 1) * P],
                        rhs=ats[dt][:, col : col + 512],
                        start=(dt == 0),
                        stop=(dt == 3),
                    )
                key = (et, tcc)
                if key not in obs:
                    obs[key] = outp.tile([P, 1024], F16, tag="ob", name="ob")
                ob = obs[key]
                nc.vector.tensor_scalar_add(
                    ob[:, half * 512 : half * 512 + 512], mp[:], pb_sb[:, et : et + 1]
                )
                if half == 1:
                    nc.sync.dma_start(
                        yT.ap()[et * P : (et + 1) * P, tcc * 1024 : (tcc + 1) * 1024],
                        ob[:],
                    )
                    del obs[key]

            # ---- K0 + Q0 before the attention corridor ----
            with tc.tile_pool(name="qkp", bufs=2, space="PSUM") as qkp:
                for ft in (4, 0):
                    for tcc in range(2):
                        pq = qkp.tile([P, 1024], F32, tag="pq", name="pq")
                        for half in range(2):
                            col = tcc * 1024 + half * 512
                            for dt in range(8):
                                nc.tensor.matmul(
                                    pq[:, half * 512 : half * 512 + 512],
                                    lhsT=wqks[dt][:, ft * P : (ft + 1) * P],
                                    rhs=xts[dt][:, col : col + 512],
                                    start=(dt == 0),
                                    stop=(dt == 7),
                                )
                        dst = qpair[ft] if ft < 4 else kts[ft - 4]
                        nc.vector.tensor_scalar_add(
                            dst[:, tcc * 1024 : (tcc + 1) * 1024],
                            pq[:],
                            qkb_sb[:, ft : ft + 1],
                        )

            # ---- attention corridor ----
            # filler chunks interleaved per iteration: (kind, args)
            fillers = {
                0: [("v", tt) for tt in range(16)],
                1: [("qk", 5, tcc, half) for tcc in range(2) for half in range(2)]
                + [("qk", 1, tcc, half) for tcc in range(2) for half in range(2)],
                2: [("qk", 6, tcc, half) for tcc in range(2) for half in range(2)]
                + [("qk", 2, tcc, half) for tcc in range(2) for half in range(2)],
                3: [("qk", 7, tcc, half) for tcc in range(2) for half in range(2)]
                + [("qk", 3, tcc, half) for tcc in range(2) for half in range(2)],
                7: [("proj", et, 0, half) for et in range(8) for half in range(2)],
            }

            with (
                tc.tile_pool(name="spool", bufs=1, space="PSUM") as spool,
                tc.tile_pool(name="opool", bufs=1, space="PSUM") as opool,
            ):
                obs = {}

                def emit_filler(f):
                    if f[0] == "v":
                        v_chunk(f[1])
                    elif f[0] == "qk":
                        qk_chunk(f[1], f[2], f[3])
                    else:
                        proj_chunk(f[1], f[2], f[3], obs)

                def emit_tail(state):
                    """Denominator + normalize for a finished iteration."""
                    po_, acc_, p_, qcc_ = state
                    for half in range(2):
                        cols = slice(half * 512, half * 512 + 512)
                        dps = dpool.tile([128, 512], F32, tag="dps", name="dps")
                        nc.tensor.matmul(
                            dps[0:64, :],
                            lhsT=ones[:],
                            rhs=acc_[:, half * 512 : half * 512 + 512],
                            start=True,
                            stop=True,
                            tile_position=(0, 0),
                        )
                        nc.tensor.matmul(
                            dps[64:128, :],
                            lhsT=ones[:],
                            rhs=acc_[:, 1024 + half * 512 : 1024 + half * 512 + 512],
                            start=True,
                            stop=True,
                            tile_position=(0, 64),
                        )
                        nc.vector.tensor_scalar_add(rb[:, cols], dps[:], 0.0)
                        nc.vector.reciprocal_approx_fast(rb[:, cols], rb[:, cols])
                    nc.vector.tensor_tensor(
                        ats[p_][:, qcc_ * 1024 : (qcc_ + 1) * 1024], po_[:], rb[:], mult
                    )

                prev = None
                iters = [(p, qcc) for p in range(4) for qcc in range(2)]
                for it, (p, qcc) in enumerate(iters):
                    fill = list(fillers.get(it, []))
                    fi = 0
                    po = opool.tile([P, 1024], F32, tag="po", name="po")
                    pend = []  # attnV emission staggered by 2 kt
                    for kt in range(16):
                        ps2 = spool.tile([P, 2048], F32, tag="ps2", name="ps2") if kt == 0 else ps2
                        # scores: row-tiled head pair, both q halves
                        for hb, rows, tp in ((0, slice(0, 64), (0, 0)), (1, slice(64, 128), (64, 0))):
                            for half in range(2):
                                qcol = qcc * 1024 + half * 512
                                nc.tensor.matmul(
                                    ps2[:, hb * 1024 + half * 512 : hb * 1024 + half * 512 + 512],
                                    lhsT=kts[p][rows, kt * P : (kt + 1) * P],
                                    rhs=qpair[p][rows, qcol : qcol + 512],
                                    start=True,
                                    stop=True,
                                    tile_position=tp,
                                )
                            nc.scalar.activation(
                                e2[:, kt, hb * 1024 : hb * 1024 + 1024],
                                ps2[:, hb * 1024 : hb * 1024 + 1024],
                                Exp,
                                scale=SCALE,
                            )
                        if kt == 0 and prev is not None:
                            emit_tail(prev)
                        # filler chunks spread over kt 2..15 (keep iteration
                        # starts free for the tail chain of the prior iter)
                        while fi * 14 < max(0, kt - 1) * len(fill):
                            emit_filler(fill[fi])
                            fi += 1
                        # denominator accumulation on DVE
                        if kt == 1:
                            nc.vector.tensor_tensor(
                                acc[:], e2[:, 0, :], e2[:, 1, :], add
                            )
                        elif kt >= 2:
                            nc.vector.tensor_tensor(acc[:], acc[:], e2[:, kt, :], add)
                        # attnV staggered 2 kt behind scores
                        pend.append(kt)
                        if len(pend) > 2:
                            emit_attnv = pend.pop(0)
                            _attnv(nc, V4, e2, po, p, emit_attnv)
                    for emit_attnv in pend:
                        _attnv(nc, V4, e2, po, p, emit_attnv)
                    prev = (po, acc, p, qcc)

                emit_tail(prev)
                # projection tcc=1 tail
                for et in range(8):
                    for half in range(2):
                        proj_chunk(et, 1, half, obs)

    nc.compile()
    return nc


def _attnv(nc, V4, e2, po, p, kt):
    for half in range(2):
        cols = slice(half * 512, half * 512 + 512)
        nc.tensor.matmul(
            po[0:64, cols],
            lhsT=V4[:, kt, 2 * p, :],
            rhs=e2[:, kt, half * 512 : half * 512 + 512],
            start=(kt == 0),
            stop=(kt == 15),
            tile_position=(0, 0),
        )
        nc.tensor.matmul(
            po[64:128, cols],
            lhsT=V4[:, kt, 2 * p + 1, :],
            rhs=e2[:, kt, 1024 + half * 512 : 1024 + half * 512 + 512],
            start=(kt == 0),
            stop=(kt == 15),
            tile_position=(0, 64),
        )


def kernel(x, qkv_w, qkv_b, proj_w, proj_b):
    global _built, LAST_RESULT
    x = np.asarray(x, np.float32)
    qkv_w = np.asarray(qkv_w, np.float32)
    qkv_b = np.asarray(qkv_b, np.float32)
    proj_w = np.asarray(proj_w, np.float32)
    proj_b = np.asarray(proj_b, np.float32)

    if _built is None:
        _built = _build()
    nc = _built

    in_maps = []
    for c in range(8):
        b, j = divmod(c, 2)
        s = j * 512
        wqkT = np.concatenate(
            [qkv_w[s : s + 512], qkv_w[1024 + s : 1024 + s + 512]]
        ).T
        in_maps.append(
            {
                "xT": np.ascontiguousarray(x[b].T).astype(np.float16),
                "wqkT": np.ascontiguousarray(wqkT).astype(np.float16),
                "wvT": np.ascontiguousarray(
                    qkv_w[2048 + s : 2048 + s + 512].T
                ).astype(np.float16),
                "wpT": np.ascontiguousarray(proj_w[:, s : s + 512].T).astype(
                    np.float16
                ),
                "qkb": np.concatenate(
                    [qkv_b[s : s + 512], qkv_b[1024 + s : 1024 + s + 512]]
                ).astype(np.float32),
                "vb": np.ascontiguousarray(qkv_b[2048 + s : 2048 + s + 512]).astype(
                    np.float16
                ),
                "pb": (proj_b if j == 0 else np.zeros_like(proj_b)).astype(
                    np.float32
                ),
            }
        )

    trace = os.environ.get("BASS_TRACE") == "1"
    if trace:
        _ensure_ntff_hook()
        try:
            import antenv.axon_hooks  # noqa: F401
        except ImportError:
            trace = False
            os.environ["BASS_NEVER_TRACE"] = "1"
    try:
        res = run_bass_kernel_spmd(nc, in_maps, core_ids=list(range(8)), trace=trace)
    except Exception:
        if not trace:
            raise
        os.environ["BASS_NEVER_TRACE"] = "1"
        res = run_bass_kernel_spmd(nc, in_maps, core_ids=list(range(8)), trace=False)
    LAST_RESULT = res

    out = np.empty((B, T, D), np.float32)
    for b in range(B):
        out[b] = (
            res.results[2 * b]["yT"].astype(np.float32)
            + res.results[2 * b + 1]["yT"].astype(np.float32)
        ).T
    return out


# revision 18
# speedup vs baseline: 1.5467x; 1.5467x over previous
"""Multi-head self-attention (B=4, T=2048, D=1024, H=16) on 8 TRN2 NeuronCores.

Sharding: core c = 2*b + j computes batch b, heads j*8..j*8+7 (tensor-parallel
over heads), and a partial projection over its 512 attention-output columns.
The host sums the two partial projections per batch. No collectives.

Per-core dataflow (all matmul inputs bf16, fp32 PSUM accumulation):
  - QK^T projection in transposed layout: psum[feat, t] = wqkT.T @ xT
  - V in natural layout [t, vfeat], stored with a ones column per head
    (V_aug[:, 64] = 1) so the attn@V matmul also produces the softmax
    denominator (row 64 of the output).
  - Transposed scores per head: s^T[k_t, q_t] = K^T_tile.T @ Q^T, exp via
    ScalarE (scale=1/8 folded in, no max subtraction: |s|*scale <~ 3).
    Heads are processed in even/odd pairs living on partitions 0:64 / 64:128
    so the K=64 matmuls pack into the PE array's row groups.
  - attn@V: out[65, q_t] = V_aug.T @ exp_s^T accumulated over k tiles;
    rows 0:64 are unnormalized head outputs, row 64 the denominator.
  - Normalize (DVE reciprocal + GpSimd partition broadcast + DVE multiply)
    into A^T[d, t] (bf16), then partial projection y^T = wpT.T @ A^T (fp32).
"""

import os

import numpy as np
import ml_dtypes

import concourse.mybir as mybir
from concourse import bacc
from concourse.tile import TileContext
from concourse.bass_utils import run_bass_kernel_spmd

B, T, D, H = 4, 2048, 1024, 16
HD = D // H
SCALE = HD**-0.5
P = 128
BF = mybir.dt.bfloat16
F32 = mybir.dt.float32
NBF = ml_dtypes.bfloat16

LAST_RESULT = None
_built = None


def _build():
    nc = bacc.Bacc("TRN2", target_bir_lowering=False, debug=False, num_devices=8)

    xT = nc.dram_tensor("xT", [D, T], BF, kind="ExternalInput")  # x[b].T
    wqkT = nc.dram_tensor("wqkT", [D, 1024], BF, kind="ExternalInput")  # (q|k).T shard
    wvT = nc.dram_tensor("wvT", [D, 512], BF, kind="ExternalInput")
    wpT = nc.dram_tensor("wpT", [512, D], BF, kind="ExternalInput")  # proj_w.T rows
    qkb = nc.dram_tensor("qkb", [1024], F32, kind="ExternalInput")
    vb = nc.dram_tensor("vb", [512], F32, kind="ExternalInput")
    pb = nc.dram_tensor("pb", [D], F32, kind="ExternalInput")
    yT = nc.dram_tensor("yT", [D, T], F32, kind="ExternalOutput")

    Exp = mybir.ActivationFunctionType.Exp
    mult = mybir.AluOpType.mult
    add = mybir.AluOpType.add

    with TileContext(nc) as tc:
        with (
            tc.tile_pool(name="pers", bufs=1) as pers,
            tc.tile_pool(name="small", bufs=1) as small,
        ):
            # ---- persistent tensors ----
            # K^T feature tiles (two heads per tile: even head rows 0:64, odd 64:128)
            kts = [pers.tile([P, T], BF, tag=f"kt{i}", name=f"kt{i}") for i in range(4)]
            # Zero-padded Q^T per head: head h's q features on rows (h%2)*64..+64,
            # zeros elsewhere, so scores matmuls contract over the full 128
            # partitions (keeps the PE activity monitor at full clock) while the
            # other head's K rows are multiplied by zero.
            qps = [pers.tile([P, T], BF, tag=f"qp{h}", name=f"qp{h}") for h in range(8)]
            # attention out, d-major, one tile per head pair so the projection
            # can start contracting early tiles before the last norm lands
            ats = [pers.tile([P, T], BF, tag=f"at{i}", name=f"at{i}") for i in range(4)]
            V4 = pers.tile([P, 16, 8, HD + 1], BF, tag="v4")  # [t-part, tt, head, 65]

            # ---- input loads ----
            ph1_cm = tc.tile_pool(name="ph1", bufs=1)
            ph1 = ph1_cm.__enter__()
            xts = []
            wqks = []
            for dt in range(8):
                t_ = ph1.tile([P, T], BF, tag=f"xt{dt}")
                nc.sync.dma_start(t_[:], xT.ap()[dt * P : (dt + 1) * P, :])
                xts.append(t_)
                w_ = ph1.tile([P, 1024], BF, tag=f"wqk{dt}")
                nc.sync.dma_start(w_[:], wqkT.ap()[dt * P : (dt + 1) * P, :])
                wqks.append(w_)
            wvs = []
            for dt in range(8):
                w_ = ph1.tile([P, 512], BF, tag=f"wv{dt}")
                nc.sync.dma_start(w_[:], wvT.ap()[dt * P : (dt + 1) * P, :])
                wvs.append(w_)
            qkb_sb = small.tile([P, 8], F32, tag="qkb")
            nc.sync.dma_start(qkb_sb[:], qkb.rearrange("(o p) -> p o", p=P))
            vb_sb = small.tile([P, 512], F32, tag="vb")
            nc.sync.dma_start(vb_sb[:], vb.ap()[None, :].to_broadcast((P, 512)))
            pb_sb = small.tile([P, 8], F32, tag="pb")
            nc.sync.dma_start(pb_sb[:], pb.rearrange("(o p) -> p o", p=P))
            # ones columns of V_aug
            nc.vector.memset(V4[:, :, :, HD : HD + 1], 1.0)
            for h in range(8):
                pad = slice(64, 128) if h % 2 == 0 else slice(0, 64)
                nc.gpsimd.memset(qps[h][pad, :], 0.0)

            with tc.tile_pool(name="ph1p", bufs=2, space="PSUM") as ph1p:
                # ---- QK^T projection: psum[feat, t] ----
                for ft in (4, 0, 5, 1, 6, 2, 7, 3):
                    for tcc in range(2):
                        pq = ph1p.tile([P, 1024], F32, tag="pq")
                        for half in range(2):
                            col = tcc * 1024 + half * 512
                            for dt in range(8):
                                nc.tensor.matmul(
                                    pq[:, half * 512 : half * 512 + 512],
                                    lhsT=wqks[dt][:, ft * P : (ft + 1) * P],
                                    rhs=xts[dt][:, col : col + 512],
                                    start=(dt == 0),
                                    stop=(dt == 7),
                                )
                        cols = slice(tcc * 1024, (tcc + 1) * 1024)
                        if ft < 4:
                            nc.vector.tensor_scalar_add(
                                qps[2 * ft][0:64, cols],
                                pq[0:64, :],
                                qkb_sb[0:64, ft : ft + 1],
                            )
                            nc.vector.tensor_scalar_add(
                                qps[2 * ft + 1][64:128, cols],
                                pq[64:128, :],
                                qkb_sb[64:128, ft : ft + 1],
                            )
                        else:
                            nc.vector.tensor_scalar_add(
                                kts[ft - 4][:, cols],
                                pq[:],
                                qkb_sb[:, ft : ft + 1],
                            )

                # ---- V: psum[t, vfeat] ----
                for tt in range(16):
                    pv = ph1p.tile([P, 512], F32, tag="pv")
                    for dt in range(8):
                        nc.tensor.matmul(
                            pv[:],
                            lhsT=xts[dt][:, tt * P : (tt + 1) * P],
                            rhs=wvs[dt][:],
                            start=(dt == 0),
                            stop=(dt == 7),
                        )
                    nc.vector.tensor_tensor(
                        V4[:, tt, :, 0:HD],
                        pv.rearrange("p (h e) -> p h e", e=HD),
                        vb_sb.rearrange("p (h e) -> p h e", e=HD),
                        mult if False else add,
                    )

            ph1_cm.__exit__(None, None, None)

            # ---- attention ----
            # Software-pipelined: iteration i emits scores+exp for (h, qcc)
            # interleaved (per kt) with iteration i-1's attn@V matmuls, so the
            # PE never drains while ACT works and vice versa.
            with (
                tc.tile_pool(name="attn", bufs=2) as attn,
                tc.tile_pool(name="norm", bufs=2) as norm,
                tc.tile_pool(name="spool", bufs=2, space="PSUM") as spool,
                tc.tile_pool(name="opool", bufs=2, space="PSUM") as opool,
            ):
                def emit_norm(state):
                    e_, po_, h_, qcc_ = state
                    rrow = norm.tile([1, 1024], F32, tag="rrow")
                    nc.vector.reciprocal(rrow[:], po_[HD : HD + 1, :])
                    rb = norm.tile([64, 1024], F32, tag="rb")
                    nc.gpsimd.partition_broadcast(rb[:], rrow[:])
                    nc.vector.tensor_tensor(
                        ats[h_ // 2][(h_ % 2) * 64 : (h_ % 2) * 64 + 64,
                                     qcc_ * 1024 : (qcc_ + 1) * 1024],
                        po_[0:HD, :],
                        rb[:],
                        mult,
                    )

                prev = None
                iters = [(h, qcc) for h in range(8) for qcc in range(2)]
                for h, qcc in iters:
                    e = attn.tile([P, 16, 1024], BF, tag="e")
                    po = opool.tile([P, 1024], F32, tag="po")
                    for kt in range(16):
                        ps = spool.tile([P, 1024], F32, tag="ps")
                        for half in range(2):
                            qcol = qcc * 1024 + half * 512
                            nc.tensor.matmul(
                                ps[:, half * 512 : half * 512 + 512],
                                lhsT=kts[h // 2][:, kt * P : (kt + 1) * P],
                                rhs=qps[h][:, qcol : qcol + 512],
                                start=True,
                                stop=True,
                            )
                        if prev is not None:
                            e_, po_, h_, _ = prev
                            for half in range(2):
                                nc.tensor.matmul(
                                    po_[0 : HD + 1, half * 512 : half * 512 + 512],
                                    lhsT=V4[:, kt, h_, :],
                                    rhs=e_[:, kt, half * 512 : half * 512 + 512],
                                    start=(kt == 0),
                                    stop=(kt == 15),
                                )
                        nc.scalar.activation(e[:, kt, :], ps[:], Exp, scale=SCALE)
                    if prev is not None:
                        emit_norm(prev)
                    prev = (e, po, h, qcc)

                # drain last iteration's attn@V + norm
                e_, po_, h_, _ = prev
                for kt in range(16):
                    for half in range(2):
                        nc.tensor.matmul(
                            po_[0 : HD + 1, half * 512 : half * 512 + 512],
                            lhsT=V4[:, kt, h_, :],
                            rhs=e_[:, kt, half * 512 : half * 512 + 512],
                            start=(kt == 0),
                            stop=(kt == 15),
                        )
                emit_norm(prev)

                # ---- projection: yT[e, t] = wpT.T @ AT ----
                # Inside the attention pool scope: psum comes from the scores
                # pool (freed by the final exps), wp/ob from small inner pools,
                # so early-et matmuls overlap the attention drain.
                with (
                    tc.tile_pool(name="ph3", bufs=1) as ph3,
                    tc.tile_pool(name="outp", bufs=3) as outp,
                ):
                    wps = []
                    for dt in range(4):
                        w_ = ph3.tile([P, D], BF, tag=f"wp{dt}")
                        nc.sync.dma_start(w_[:], wpT.ap()[dt * P : (dt + 1) * P, :])
                        wps.append(w_)
                    for et in range(8):
                        for tcc in range(2):
                            pp = spool.tile([P, 1024], F32, tag="ps")
                            for half in range(2):
                                col = tcc * 1024 + half * 512
                                for dt in range(4):
                                    nc.tensor.matmul(
                                        pp[:, half * 512 : half * 512 + 512],
                                        lhsT=wps[dt][:, et * P : (et + 1) * P],
                                        rhs=ats[dt][:, col : col + 512],
                                        start=(dt == 0),
                                        stop=(dt == 3),
                                    )
                            ob = outp.tile([P, 1024], F32, tag="ob")
                            nc.vector.tensor_scalar_add(ob[:], pp[:], pb_sb[:, et : et + 1])
                            nc.sync.dma_start(
                                yT.ap()[et * P : (et + 1) * P, tcc * 1024 : (tcc + 1) * 1024],
                                ob[:],
                            )

    nc.compile()
    return nc


def kernel(x, qkv_w, qkv_b, proj_w, proj_b):
    global _built, LAST_RESULT
    x = np.asarray(x, np.float32)
    qkv_w = np.asarray(qkv_w, np.float32)
    qkv_b = np.asarray(qkv_b, np.float32)
    proj_w = np.asarray(proj_w, np.float32)
    proj_b = np.asarray(proj_b, np.float32)

    if _built is None:
        _built = _build()
    nc = _built

    in_maps = []
    for c in range(8):
        b, j = divmod(c, 2)
        s = j * 512
        wqkT = np.concatenate([qkv_w[s : s + 512], qkv_w[1024 + s : 1024 + s + 512]]).T
        in_maps.append(
            {
                "xT": np.ascontiguousarray(x[b].T).astype(NBF),
                "wqkT": np.ascontiguousarray(wqkT).astype(NBF),
                "wvT": np.ascontiguousarray(qkv_w[2048 + s : 2048 + s + 512].T).astype(NBF),
                "wpT": np.ascontiguousarray(proj_w[:, s : s + 512].T).astype(NBF),
                "qkb": np.concatenate([qkv_b[s : s + 512], qkv_b[1024 + s : 1024 + s + 512]]),
                "vb": np.ascontiguousarray(qkv_b[2048 + s : 2048 + s + 512]),
                "pb": proj_b if j == 0 else np.zeros_like(proj_b),
            }
        )

    trace = os.environ.get("BASS_TRACE") == "1"
    if trace:
        try:
            import antenv.axon_hooks  # noqa: F401  (needed by the axon trace path)
        except ImportError:
            trace = False
            os.environ["BASS_NEVER_TRACE"] = "1"
    res = run_bass_kernel_spmd(nc, in_maps, core_ids=list(range(8)), trace=trace)
    LAST_RESULT = res

    out = np.empty((B, T, D), np.float32)
    for b in range(B):
        out[b] = (res.results[2 * b]["yT"] + res.results[2 * b + 1]["yT"]).T
    return out



# revision 19
# speedup vs baseline: 1.5650x; 1.0119x over previous
"""Multi-head self-attention (B=4, T=2048, D=1024, H=16) on 8 TRN2 NeuronCores.

Sharding: core c = 2*b + j computes batch b, heads j*8..j*8+7 (tensor-parallel
over heads), and a partial projection over its 512 attention-output columns.
The host sums the two partial projections per batch. No collectives.

Per-core dataflow (fp16 operands, fp32 PSUM):
  - Scores in transposed layout s^T[k, q] with PE row-tiling: the two heads
    of a pair contract K=64 each on array rows 0:64 / 64:128 concurrently
    (tile_position (0,0)/(64,0)) -> 2x score throughput vs zero-padding.
  - One 2048-wide exp per k-tile on ScalarE covers both heads (scale=1/8).
  - attn@V with PE col-tiling: the pair's V matmuls write po[0:64]/po[64:128]
    concurrently (tile_position (0,0)/(0,64)) -> 2x vs M=65 serial.
  - Softmax denominators: DVE chain-sums the 16 exp tiles (fp16, 2x mode),
    then two M=1 ones-matmuls (col positions 0/32) reduce partitions,
    reciprocal_approx_fast + gpsimd partition_broadcast + one DVE multiply
    normalize into ats (fp16).
  - QKV / V / projection matmul chunks are interleaved into the attention
    iterations so the PE uses the slack under the ScalarE-bound exp corridor.
"""

import os

import numpy as np

import concourse.mybir as mybir
from concourse import bacc, bass_isa
from concourse.tile import TileContext
from concourse.bass_utils import run_bass_kernel_spmd

B, T, D, H = 4, 2048, 1024, 16
HD = D // H
SCALE = HD**-0.5
P = 128
F16 = mybir.dt.float16
F32 = mybir.dt.float32

LAST_RESULT = None
_built = None


def _ensure_ntff_hook():
    """Install the axon NTFF profile hook if the env lacks antenv.axon_hooks."""
    try:
        import antenv.axon_hooks  # noqa: F401

        return
    except ImportError:
        pass
    try:
        import sys
        import types

        import antenv
        from trn_agent_boot.trn_boot import _ntff_profile_via_ctypes

        hook = _ntff_profile_via_ctypes("/opt/axon/libaxon_pjrt.so")
        if hook is None:
            return
        mod = types.ModuleType("antenv.axon_hooks")
        mod._hook = hook
        mod.get_axon_ntff_profile_hook = lambda: mod._hook

        def _set(h):
            mod._hook = h

        mod.set_axon_ntff_profile_hook = _set
        sys.modules["antenv.axon_hooks"] = mod
        antenv.axon_hooks = mod
    except Exception:
        pass


def _build():
    nc = bacc.Bacc("TRN2", target_bir_lowering=False, debug=False, num_devices=8)

    xT = nc.dram_tensor("xT", [D, T], F16, kind="ExternalInput")  # x[b].T
    wqkT = nc.dram_tensor("wqkT", [D, 1024], F16, kind="ExternalInput")  # (q|k).T
    wvT = nc.dram_tensor("wvT", [D, 512], F16, kind="ExternalInput")
    wpT = nc.dram_tensor("wpT", [512, D], F16, kind="ExternalInput")
    qkb = nc.dram_tensor("qkb", [1024], F32, kind="ExternalInput")
    vb = nc.dram_tensor("vb", [512], F16, kind="ExternalInput")
    pb = nc.dram_tensor("pb", [D], F32, kind="ExternalInput")
    yT = nc.dram_tensor("yT", [D, T], F16, kind="ExternalOutput")

    Exp = mybir.ActivationFunctionType.Exp
    mult = mybir.AluOpType.mult
    add = mybir.AluOpType.add

    with TileContext(nc) as tc:
        with (
            tc.tile_pool(name="pers", bufs=1) as pers,
            tc.tile_pool(name="e2p", bufs=1) as e2p,
            tc.tile_pool(name="ph1", bufs=1) as ph1,
            tc.tile_pool(name="ph3", bufs=1) as ph3,
            tc.tile_pool(name="outp", bufs=1) as outp,
            tc.tile_pool(name="mpool", bufs=1, space="PSUM") as mpool,
            tc.tile_pool(name="dpool", bufs=1, space="PSUM") as dpool,
        ):
            # ---- persistent tensors ----
            # K^T per pair: even head feats on partitions 0:64, odd on 64:128
            kts = [pers.tile([P, T], F16, tag=f"kt{i}", name=f"kt{i}") for i in range(4)]
            # Q^T per pair, same partition split
            qpair = [pers.tile([P, T], F16, tag=f"qp{i}", name=f"qp{i}") for i in range(4)]
            # normalized attention out per pair (rows 0:64 even, 64:128 odd)
            ats = [pers.tile([P, T], F16, tag=f"at{i}", name=f"at{i}") for i in range(4)]
            V4 = pers.tile([P, 16, 8, HD], F16, tag="v4", name="v4")  # [t-part, tt, head, hd]
            acc = pers.tile([P, 1024], F16, tag="acc", name="acc")  # denom partials (A|B cols)
            rb = pers.tile([P, 512], F32, tag="rb", name="rb")  # 1/d broadcast
            ones = pers.tile([P, 64], F16, tag="ones", name="ones")
            qkb_sb = pers.tile([P, 8], F32, tag="qkb", name="qkb")
            vb_sb = pers.tile([P, 512], F16, tag="vb", name="vb")
            pb_sb = pers.tile([P, 8], F32, tag="pb", name="pb")
            # e2[:, kt, 0:1024] = exp scores head A, [:, kt, 1024:2048] head B
            e2 = e2p.tile([P, 16, 1024], F16, tag="e2", name="e2")

            # ---- input loads ----
            xts = []
            wqks = []
            for dt in range(8):
                w_ = ph1.tile([P, 1024], F16, tag=f"wqk{dt}", name=f"wqk{dt}")
                nc.sync.dma_start(w_[:], wqkT.ap()[dt * P : (dt + 1) * P, :])
                wqks.append(w_)
            for dt in range(8):
                t_ = ph1.tile([P, T], F16, tag=f"xt{dt}", name=f"xt{dt}")
                nc.sync.dma_start(t_[:], xT.ap()[dt * P : (dt + 1) * P, :])
                xts.append(t_)
            wvs = []
            for dt in range(8):
                w_ = ph1.tile([P, 512], F16, tag=f"wv{dt}", name=f"wv{dt}")
                nc.sync.dma_start(w_[:], wvT.ap()[dt * P : (dt + 1) * P, :])
                wvs.append(w_)
            wps = []
            for dt in range(4):
                w_ = ph3.tile([P, D], F16, tag=f"wp{dt}", name=f"wp{dt}")
                nc.sync.dma_start(w_[:], wpT.ap()[dt * P : (dt + 1) * P, :])
                wps.append(w_)
            nc.sync.dma_start(qkb_sb[:], qkb.rearrange("(o p) -> p o", p=P))
            nc.sync.dma_start(vb_sb[:], vb.ap()[None, :].to_broadcast((P, 512)))
            nc.sync.dma_start(pb_sb[:], pb.rearrange("(o p) -> p o", p=P))
            nc.vector.memset(ones[:], 1.0)
            # warm the Exp table while phase 1 runs
            nc.scalar.activation(rb[0:1, 0:1], qkb_sb[0:1, 0:1], Exp, scale=0.0)

            # ---- helper: QK projection chunk (ft, tcc, half) ----
            # ft 0..3 -> qpair[ft]; ft 4..7 -> kts[ft-4]
            def qk_chunk(ft, tcc, half):
                mp = mpool.tile([P, 512], F32, tag="mp", name="mp")
                col = tcc * 1024 + half * 512
                for dt in range(8):
                    nc.tensor.matmul(
                        mp[:],
                        lhsT=wqks[dt][:, ft * P : (ft + 1) * P],
                        rhs=xts[dt][:, col : col + 512],
                        start=(dt == 0),
                        stop=(dt == 7),
                    )
                dst = qpair[ft] if ft < 4 else kts[ft - 4]
                nc.vector.tensor_scalar_add(
                    dst[:, col : col + 512], mp[:], qkb_sb[:, ft : ft + 1]
                )

            def v_chunk(tt):
                mp = mpool.tile([P, 512], F32, tag="mp", name="mp")
                for dt in range(8):
                    nc.tensor.matmul(
                        mp[:],
                        lhsT=xts[dt][:, tt * P : (tt + 1) * P],
                        rhs=wvs[dt][:],
                        start=(dt == 0),
                        stop=(dt == 7),
                    )
                nc.vector.tensor_tensor(
                    V4[:, tt, :, :],
                    mp.rearrange("p (h e) -> p h e", e=HD),
                    vb_sb.rearrange("p (h e) -> p h e", e=HD),
                    add,
                )

            def proj_chunk(et, qb):
                mp = mpool.tile([P, 512], F32, tag="mp", name="mp")
                col = qb * 512
                for dt in range(4):
                    nc.tensor.matmul(
                        mp[:],
                        lhsT=wps[dt][:, et * P : (et + 1) * P],
                        rhs=ats[dt][:, col : col + 512],
                        start=(dt == 0),
                        stop=(dt == 3),
                    )
                ob = outp.tile([P, 512], F16, tag="ob", name="ob")
                nc.vector.tensor_scalar_add(ob[:], mp[:], pb_sb[:, et : et + 1])
                nc.sync.dma_start(
                    yT.ap()[et * P : (et + 1) * P, col : col + 512], ob[:]
                )

            # ---- K0 + Q0 before the attention corridor ----
            with tc.tile_pool(name="qkp", bufs=2, space="PSUM") as qkp:
                for ft in (4, 0):
                    for tcc in range(2):
                        pq = qkp.tile([P, 1024], F32, tag="pq", name="pq")
                        for half in range(2):
                            col = tcc * 1024 + half * 512
                            for dt in range(8):
                                nc.tensor.matmul(
                                    pq[:, half * 512 : half * 512 + 512],
                                    lhsT=wqks[dt][:, ft * P : (ft + 1) * P],
                                    rhs=xts[dt][:, col : col + 512],
                                    start=(dt == 0),
                                    stop=(dt == 7),
                                )
                        dst = qpair[ft] if ft < 4 else kts[ft - 4]
                        nc.vector.tensor_scalar_add(
                            dst[:, tcc * 1024 : (tcc + 1) * 1024],
                            pq[:],
                            qkb_sb[:, ft : ft + 1],
                        )

            # ---- attention corridor ----
            # filler chunks interleaved per iteration: (kind, args)
            fillers = {
                0: [("v", tt) for tt in range(16)],
                2: [("qk", 5, tcc, half) for tcc in range(2) for half in range(2)],
                3: [("qk", 1, tcc, half) for tcc in range(2) for half in range(2)],
                4: [("qk", 6, tcc, half) for tcc in range(2) for half in range(2)],
                5: [("qk", 2, tcc, half) for tcc in range(2) for half in range(2)],
                6: [("qk", 7, tcc, half) for tcc in range(2) for half in range(2)],
                7: [("qk", 3, tcc, half) for tcc in range(2) for half in range(2)],
                13: [("proj", et, 0) for et in range(8)],
                14: [("proj", et, 1) for et in range(8)],
                15: [("proj", et, 2) for et in range(8)],
            }

            with (
                tc.tile_pool(name="spool", bufs=2, space="PSUM") as spool,
                tc.tile_pool(name="opool", bufs=2, space="PSUM") as opool,
            ):
                def emit_filler(f):
                    if f[0] == "v":
                        v_chunk(f[1])
                    elif f[0] == "qk":
                        qk_chunk(f[1], f[2], f[3])
                    else:
                        proj_chunk(f[1], f[2])

                def emit_tail(state):
                    """Denominator + normalize for a finished iteration."""
                    po_, acc_, p_, qb_ = state
                    dps = dpool.tile([128, 512], F32, tag="dps", name="dps")
                    nc.tensor.matmul(
                        dps[0:64, :], lhsT=ones[:], rhs=acc_[:, 0:512],
                        start=True, stop=True, tile_position=(0, 0),
                    )
                    nc.tensor.matmul(
                        dps[64:128, :], lhsT=ones[:], rhs=acc_[:, 512:1024],
                        start=True, stop=True, tile_position=(0, 64),
                    )
                    nc.vector.tensor_scalar_add(rb[:], dps[:], 0.0)
                    nc.vector.reciprocal_approx_fast(rb[:], rb[:])
                    nc.vector.tensor_tensor(
                        ats[p_][:, qb_ * 512 : qb_ * 512 + 512], po_[:], rb[:], mult
                    )

                prev = None
                iters = [(p, qb) for p in range(4) for qb in range(4)]
                for it, (p, qb) in enumerate(iters):
                    fill = list(fillers.get(it, []))
                    fi = 0
                    po = opool.tile([P, 512], F32, tag="po", name="po")
                    pend = []
                    for kt in range(16):
                        ps = spool.tile([P, 1024], F32, tag="ps", name="ps")
                        qcol = qb * 512
                        nc.tensor.matmul(
                            ps[:, 0:512],
                            lhsT=kts[p][0:64, kt * P : (kt + 1) * P],
                            rhs=qpair[p][0:64, qcol : qcol + 512],
                            start=True, stop=True, tile_position=(0, 0),
                        )
                        nc.tensor.matmul(
                            ps[:, 512:1024],
                            lhsT=kts[p][64:128, kt * P : (kt + 1) * P],
                            rhs=qpair[p][64:128, qcol : qcol + 512],
                            start=True, stop=True, tile_position=(64, 0),
                        )
                        nc.scalar.activation(e2[:, kt, :], ps[:], Exp, scale=SCALE)
                        if kt == 0 and prev is not None:
                            emit_tail(prev)
                        while fi * 14 < max(0, kt - 1) * len(fill):
                            emit_filler(fill[fi])
                            fi += 1
                        if kt == 1:
                            nc.vector.tensor_tensor(acc[:], e2[:, 0, :], e2[:, 1, :], add)
                        elif kt >= 2:
                            nc.vector.tensor_tensor(acc[:], acc[:], e2[:, kt, :], add)
                        pend.append(kt)
                        if len(pend) > 2:
                            _attnv(nc, V4, e2, po, p, pend.pop(0))
                    for k2 in pend:
                        _attnv(nc, V4, e2, po, p, k2)
                    prev = (po, acc, p, qb)

                emit_tail(prev)
                for et in range(8):
                    proj_chunk(et, 3)

    nc.compile()
    return nc


def _attnv(nc, V4, e2, po, p, kt):
    nc.tensor.matmul(
        po[0:64, :],
        lhsT=V4[:, kt, 2 * p, :],
        rhs=e2[:, kt, 0:512],
        start=(kt == 0),
        stop=(kt == 15),
        tile_position=(0, 0),
    )
    nc.tensor.matmul(
        po[64:128, :],
        lhsT=V4[:, kt, 2 * p + 1, :],
        rhs=e2[:, kt, 512:1024],
        start=(kt == 0),
        stop=(kt == 15),
        tile_position=(0, 64),
    )


def kernel(x, qkv_w, qkv_b, proj_w, proj_b):
    global _built, LAST_RESULT
    x = np.asarray(x, np.float32)
    qkv_w = np.asarray(qkv_w, np.float32)
    qkv_b = np.asarray(qkv_b, np.float32)
    proj_w = np.asarray(proj_w, np.float32)
    proj_b = np.asarray(proj_b, np.float32)

    if _built is None:
        _built = _build()
    nc = _built

    in_maps = []
    for c in range(8):
        b, j = divmod(c, 2)
        s = j * 512
        wqkT = np.concatenate(
            [qkv_w[s : s + 512], qkv_w[1024 + s : 1024 + s + 512]]
        ).T
        in_maps.append(
            {
                "xT": np.ascontiguousarray(x[b].T).astype(np.float16),
                "wqkT": np.ascontiguousarray(wqkT).astype(np.float16),
                "wvT": np.ascontiguousarray(
                    qkv_w[2048 + s : 2048 + s + 512].T
                ).astype(np.float16),
                "wpT": np.ascontiguousarray(proj_w[:, s : s + 512].T).astype(
                    np.float16
                ),
                "qkb": np.concatenate(
                    [qkv_b[s : s + 512], qkv_b[1024 + s : 1024 + s + 512]]
                ).astype(np.float32),
                "vb": np.ascontiguousarray(qkv_b[2048 + s : 2048 + s + 512]).astype(
                    np.float16
                ),
                "pb": (proj_b if j == 0 else np.zeros_like(proj_b)).astype(
                    np.float32
                ),
            }
        )

    trace = os.environ.get("BASS_TRACE") == "1"
    if trace:
        _ensure_ntff_hook()
        try:
            import antenv.axon_hooks  # noqa: F401
        except ImportError:
            trace = False
            os.environ["BASS_NEVER_TRACE"] = "1"
    try:
        res = run_bass_kernel_spmd(nc, in_maps, core_ids=list(range(8)), trace=trace)
    except Exception:
        if not trace:
            raise
        os.environ["BASS_NEVER_TRACE"] = "1"
        res = run_bass_kernel_spmd(nc, in_maps, core_ids=list(range(8)), trace=False)
    LAST_RESULT = res

    out = np.empty((B, T, D), np.float32)
    for b in range(B):
        out[b] = (
            res.results[2 * b]["yT"].astype(np.float32)
            + res.results[2 * b + 1]["yT"].astype(np.float32)
        ).T
    return out
